# revision 20
# baseline (speedup 1.0000x reference)
"""BertAttention (QKV + MHA + output proj + residual + LayerNorm) on 8 TRN2 cores.

v3: reworked from v2 for engine balance and fewer instructions.
- Scores run from the natural [feature, token] q/k layout with PE row tiling
  (head A rows 0-63 @ tile (0,0), head B rows 64-127 @ (64,0)) — the v2
  q8/k8 SBUF->SBUF re-layout DMAs are gone.
- exp is split between ACT (fp8 out) and DVE (Schraudolph bit-trick:
  tensor_scalar f32->uint8 with A=8*scale/ln2, B=56.13, bitcast to e4m3).
- Context matmul is fp8 DoubleRow over j-chunk pairs (va [128, 2, 80] planes,
  es [128, 2, 512]), halving PE instructions for the PV product.
- x8 is loaded to SBUF once for both batches.
- Softmax normalize reads ctx PSUM directly (reciprocal + broadcast + mul).
- LayerNorm mean rides a DoubleRow matmul with row-summed Wo (wsum) instead
  of the per-chunk bf16 copy + ones matmul chain.

Sharding: heads 2c,2c+1 -> core c (tensor-parallel attention); output
projection + LayerNorm token-sharded (512 flat tokens per core) after an
on-device AllToAll of the fp8 per-head context.
"""
import sys

sys.path.insert(0, "/opt/trn_rl_repo")

import numpy as np
import ml_dtypes

import concourse.bacc as bacc
import concourse.mybir as mybir
import concourse.tile as tile
from concourse.bass_utils import run_bass_kernel_spmd
from concourse.masks import make_identity

B, S, H = 2, 2048, 1024
NH, HD = 16, 64
W = 8                    # cores
T = B * S                # 4096 flat tokens
TOK = T // W             # 512 tokens owned per core
CPC = (NH // W) * HD     # 128 ctx channels per core (2 heads)
QT = 512                 # query tile (matmul free dim)
NQT = S // QT            # 4 query tiles per batch
NKC = S // 128           # 16 key chunks per batch
NP = NKC // 2            # 8 key-chunk pairs per batch
NK = H // 128            # 8 contraction chunks for the projections
NKP = NK // 2            # 4 DoubleRow pair-chunks

F32 = mybir.dt.float32
BF16 = mybir.dt.bfloat16
FP8 = mybir.dt.float8e4
U8 = mybir.dt.uint8
BF = ml_dtypes.bfloat16
E4 = ml_dtypes.float8_e4m3
DR = mybir.MatmulPerfMode.DoubleRow

WS = 64.0                # host-side weight prescale
EXP_SCALE = 0.125 / (WS * WS)
LN2 = 0.6931471805599453
A_DVE = 8.0 * EXP_SCALE / LN2   # Schraudolph slope for e4m3 bits
B_DVE = 55.63                   # round-optimal bias (HW DVE converts by rnd)

_NC_CACHE = {}

PHASE_MARKS = []


def _mark(nc, name):
    PHASE_MARKS.append((name, int(nc.next_id())))


def build_nc(no_collective=False, reps=1, has_mask=False,
             dve_pairs=(3, 6), es_bufs=6, sp_bufs=2, **_ignored):
    PHASE_MARKS.clear()
    nc = bacc.Bacc(None)

    x8 = nc.dram_tensor("x8", [H, T], FP8, kind="ExternalInput")
    wq = nc.dram_tensor("wq", [128, NKP, 2, CPC], FP8, kind="ExternalInput")
    wk = nc.dram_tensor("wk", [128, NKP, 2, CPC], FP8, kind="ExternalInput")
    wv = nc.dram_tensor("wv", [128, NKP, 2, CPC], FP8, kind="ExternalInput")
    bq = nc.dram_tensor("bq", [CPC], F32, kind="ExternalInput")   # 64*bq
    bk = nc.dram_tensor("bk", [CPC], F32, kind="ExternalInput")   # 64*bk
    bv = nc.dram_tensor("bv", [CPC], F32, kind="ExternalInput")   # bv
    wo = nc.dram_tensor("wo", [128, NKP, 2, H], FP8, kind="ExternalInput")
    wsum = nc.dram_tensor("wsum", [128, NKP, 2, 16], FP8, kind="ExternalInput")
    obg_in = nc.dram_tensor("obg_in", [128, NK, 2], F32, kind="ExternalInput")
    h2 = nc.dram_tensor("h2", [128, NK, TOK], F32, kind="ExternalInput")
    h2s = nc.dram_tensor("h2s", [1, TOK], F32, kind="ExternalInput")  # mean_h h2
    if has_mask:
        maskT = nc.dram_tensor("maskT", [B, S], F32, kind="ExternalInput")
    y = nc.dram_tensor("y", [H, TOK], F32, kind="ExternalOutput")

    from contextlib import ExitStack
    with tile.TileContext(nc) as tc, ExitStack() as _stk:
        constp = _stk.enter_context(tc.tile_pool(name="const", bufs=1))
        wpool = _stk.enter_context(tc.tile_pool(name="weights", bufs=1))
        xtp = _stk.enter_context(tc.tile_pool(name="xt", bufs=1))
        qkp = _stk.enter_context(tc.tile_pool(name="qk", bufs=2))
        vsp = _stk.enter_context(tc.tile_pool(name="vstage", bufs=2))
        vap = _stk.enter_context(tc.tile_pool(name="vaug", bufs=2))
        expp = _stk.enter_context(tc.tile_pool(name="exps", bufs=es_bufs))
        zp = _stk.enter_context(tc.tile_pool(name="znorm", bufs=2))
        stp = _stk.enter_context(tc.tile_pool(name="stage", bufs=4))
        dramp = _stk.enter_context(tc.tile_pool(name="dram", bufs=1, space="DRAM"))
        lnp = _stk.enter_context(tc.tile_pool(name="ln", bufs=1))
        xtl = _stk.enter_context(tc.tile_pool(name="xtile", bufs=NK))
        cop = _stk.enter_context(tc.tile_pool(name="ctxown", bufs=1))
        # ---- constants ----
        ident = constp.tile([128, 128], BF16)
        make_identity(nc, ident[:])
        ones128 = constp.tile([128, 1], BF16)
        nc.vector.memset(ones128[:], 1.0 / H)  # mom2 matmul produces mean(x^2)
        # per-batch a2a buffers: core c owns tokens [c*HT,(c+1)*HT) of EACH
        # batch; batch-b tile t sends halves to cores 2t and 2t+1.  The
        # batch-0 collective + half the tail overlap batch-1 attention.
        HT = TOK // 2
        a2a_ins, a2a_outs = [], []
        for _b in range(B):
            a2a_in_t = dramp.tile([W, CPC, HT], FP8, tag=f"a2ai{_b}")
            a2a_out_t = dramp.tile([W, CPC, HT], FP8, tag=f"a2ao{_b}")
            a2a_ins.append(a2a_in_t)
            a2a_outs.append(a2a_out_t)

        for rep in range(reps):
            # ---- small inputs / weights (reloaded per rep) ----
            # order matters: QKV t=0 needs wq/wk/wv + the first xt chunks
            wq_sb = wpool.tile([128, NKP, 2, CPC], FP8, tag="wq", bufs=1)
            wk_sb = wpool.tile([128, NKP, 2, CPC], FP8, tag="wk", bufs=1)
            wv_sb = wpool.tile([128, NKP, 2, CPC], FP8, tag="wv", bufs=1)
            for w_dram, w_sb in ((wq, wq_sb), (wk, wk_sb), (wv, wv_sb)):
                nc.sync.dma_start(out=w_sb[:, :, :, :], in_=w_dram[:, :, :, :])
            # x8 resident for both batches: [128, (p,i), T]
            xt_tiles = []
            for p in range(NKP):
                xt_t = xtp.tile([128, 2, T], FP8, tag=f"xt{p}",
                                bufs=min(reps, 2))
                for i in range(2):
                    r0 = (2 * p + i) * 128
                    nc.sync.dma_start(
                        out=xt_t[:, i, 0:1024],
                        in_=x8[r0:r0 + 128, 0:1024],
                    )
                xt_tiles.append(xt_t)
            biases = constp.tile([128, 3], F32, tag="biases", bufs=min(reps, 2))
            nc.sync.dma_start(out=biases[:, 0:1], in_=bq[:].unsqueeze(1))
            nc.sync.dma_start(out=biases[:, 1:2], in_=bk[:].unsqueeze(1))
            nc.sync.dma_start(out=biases[:, 2:3], in_=bv[:].unsqueeze(1))
            for p in range(NKP):
                for i in range(2):
                    r0 = (2 * p + i) * 128
                    nc.sync.dma_start(
                        out=xt_tiles[p][:, i, 1024:T],
                        in_=x8[r0:r0 + 128, 1024:T],
                    )
            if has_mask:
                mask_sb = constp.tile([128, B, NKC], F32, tag="mask",
                                      bufs=min(reps, 2))
                nc.sync.dma_start(
                    out=mask_sb[:, :, :],
                    in_=maskT.rearrange("b (j p) -> p b j", p=128),
                )
            wo_sb = wpool.tile([128, NKP, 2, H], FP8, tag="wo", bufs=1)
            wsum_sb = wpool.tile([128, NKP, 2, 16], FP8, tag="wsum", bufs=1)
            h2_sb = wpool.tile([128, NK, TOK], F32, tag="h2", bufs=1)
            h2s_sb = wpool.tile([1, TOK], F32, tag="h2s", bufs=1)
            obg = constp.tile([128, NK, 2], F32, tag="obg", bufs=min(reps, 2))

            with ExitStack() as _ps_stk:
                qkv_ps = _ps_stk.enter_context(
                    tc.tile_pool(name=f"qkv_ps{rep}", bufs=2, space="PSUM"))
                sc_ps = _ps_stk.enter_context(
                    tc.tile_pool(name=f"sc_ps{rep}", bufs=sp_bufs, space="PSUM"))
                ctx_ps = _ps_stk.enter_context(
                    tc.tile_pool(name=f"ctx_ps{rep}", bufs=2, space="PSUM"))
                for b in range(B):
                    bsl = slice(b * S, (b + 1) * S)
                    _mark(nc, f"qkv_b{b}")
                    # -------- QKV projections (fp8 DoubleRow) --------
                    qnat = qkp.tile([128, S], FP8, tag="qnat")
                    knat = qkp.tile([128, S], FP8, tag="knat")
                    vaA = vap.tile([128, NP, 2, 80], FP8, tag="vaA")
                    vaB = vap.tile([128, NP, 2, 80], FP8, tag="vaB")
                    for va in (vaA, vaB):
                        nc.gpsimd.memset(va[:, :, :, 65:80], 0.0)
                        nc.gpsimd.memset(va[:, :, :, 64:65], 1.0)
                    for t in range(NQT):
                        tsl = slice(t * QT, (t + 1) * QT)
                        gsl = slice(b * S + t * QT, b * S + (t + 1) * QT)
                        for w_sb, bcol, dstT in (
                            (wq_sb, 0, qnat), (wk_sb, 1, knat), (wv_sb, 2, None)
                        ):
                            ps = qkv_ps.tile([128, QT], F32, tag="qkv")
                            for p in range(NKP):
                                nc.tensor.matmul(
                                    ps[:, :],
                                    w_sb[:, p, :, :],
                                    xt_tiles[p][:, :, gsl],
                                    start=(p == 0),
                                    stop=(p == NKP - 1),
                                    perf_mode=DR,
                                )
                            if dstT is not None:
                                # q/k carry the x64 scale; bias pre-scaled.
                                # batch 0 evacuates on ACT (idle before the
                                # first exp); batch 1 on DVE (ACT is busy)
                                if b == 0:
                                    nc.scalar.activation(
                                        dstT[:, tsl], ps[:, :],
                                        mybir.ActivationFunctionType.Identity,
                                        bias=biases[:, bcol:bcol + 1],
                                        scale=1.0,
                                    )
                                else:
                                    nc.vector.tensor_scalar_add(
                                        dstT[:, tsl], ps[:, :],
                                        biases[:, bcol:bcol + 1],
                                    )
                            else:
                                vst = vsp.tile([128, QT], BF16, tag="vst")
                                if b == 0:
                                    nc.scalar.activation(
                                        vst[:, :], ps[:, :],
                                        mybir.ActivationFunctionType.Identity,
                                        bias=biases[:, 2:3],
                                        scale=1.0 / WS,
                                    )
                                else:
                                    nc.vector.tensor_scalar(
                                        vst[:, :], ps[:, :],
                                        1.0 / WS, biases[:, 2:3],
                                        op0=mybir.AluOpType.mult,
                                        op1=mybir.AluOpType.add,
                                    )
                                for pm in range(2):  # key-chunk pairs in tile
                                    m = 2 * t + pm
                                    vps = qkv_ps.tile(
                                        [128, 2, 128], BF16, tag="qkv"
                                    )
                                    for mm in range(2):
                                        s4 = 2 * pm + mm
                                        nc.tensor.transpose(
                                            vps[:, mm, :],
                                            vst[:, s4 * 128:(s4 + 1) * 128],
                                            ident[:, :],
                                        )
                                    nc.vector.tensor_copy(
                                        vaA[:, m, :, 0:64], vps[:, :, 0:64]
                                    )
                                    nc.vector.tensor_copy(
                                        vaB[:, m, :, 0:64], vps[:, :, 64:128]
                                    )

                    _mark(nc, f"attn_b{b}")
                    # -------- attention --------
                    for t in range(NQT):
                        tsl = slice(t * QT, (t + 1) * QT)
                        cpA = ctx_ps.tile([80, QT], F32, tag="ctx")
                        cpB = ctx_ps.tile([80, QT], F32, tag="ctx")
                        for m in range(NP):
                            sps = []
                            for h, cp in ((0, cpA), (1, cpB)):
                                hsl = slice(64 * h, 64 * h + 64)
                                sp = sc_ps.tile([128, 2, QT], F32, tag="sc")
                                for mm in range(2):
                                    j = 2 * m + mm
                                    jsl = slice(j * 128, (j + 1) * 128)
                                    nc.tensor.matmul(
                                        sp[:, mm, :],
                                        knat[hsl, jsl],
                                        qnat[hsl, tsl],
                                        start=True, stop=True,
                                        tile_position=(64 * h, 0),
                                    )
                                sps.append(sp)
                            for h, (cp, va, sp) in enumerate(
                                ((cpA, vaA, sps[0]), (cpB, vaB, sps[1]))
                            ):
                                es = expp.tile([128, 2, QT], FP8, tag="es")
                                if has_mask:
                                    for mm in range(2):
                                        j = 2 * m + mm
                                        nc.scalar.activation(
                                            es[:, mm, :], sp[:, mm, :],
                                            mybir.ActivationFunctionType.Exp,
                                            bias=mask_sb[:, b, j:j + 1],
                                            scale=EXP_SCALE,
                                        )
                                elif m in dve_pairs:
                                    nc.vector.tensor_scalar(
                                        es[:, :, :].bitcast(U8), sp[:, :, :],
                                        A_DVE, B_DVE,
                                        op0=mybir.AluOpType.mult,
                                        op1=mybir.AluOpType.add,
                                    )
                                else:
                                    nc.scalar.activation(
                                        es[:, :, :], sp[:, :, :],
                                        mybir.ActivationFunctionType.Exp,
                                        scale=EXP_SCALE,
                                    )
                                nc.tensor.matmul(
                                    cp[:, :],
                                    va[:, m, :, :],
                                    es[:, :, :],
                                    start=(m == 0), stop=(m == NP - 1),
                                    perf_mode=DR,
                                )
                        # normalize: z must bounce through SBUF — custom-DVE
                        # reciprocal misreads PSUM sources on HW
                        for half_i, cp in ((0, cpA), (1, cpB)):
                            zr = zp.tile([1, QT], F32, tag="zr")
                            nc.vector.tensor_copy(zr[:, :], cp[64:65, :])
                            z = zp.tile([1, QT], F32, tag="z")
                            nc.vector.reciprocal_approx_fast(
                                z[:, :], zr[:, :]
                            )
                            rb = zp.tile([64, QT], F32, tag="rb")
                            nc.gpsimd.partition_broadcast(
                                rb[:, :], z[:, :], channels=64
                            )
                            st = stp.tile([64, QT], FP8, tag="st")
                            nc.vector.tensor_mul(
                                st[:, :], cp[0:64, :], rb[:, :]
                            )
                            for half in range(2):
                                hsl2 = slice(half * HT, (half + 1) * HT)
                                nc.gpsimd.dma_start(
                                    out=a2a_ins[b][2 * t + half,
                                                   64 * half_i:64 * half_i + 64,
                                                   :],
                                    in_=st[:, hsl2],
                                )

                    if b == 0:
                        _mark(nc, "a2aA")
                        if no_collective:
                            for i in range(W):
                                nc.sync.dma_start(
                                    out=a2a_outs[0][i, :, :],
                                    in_=a2a_ins[0][i, :, :],
                                )
                        else:
                            nc.gpsimd.collective_compute(
                                "AllToAll",
                                mybir.AluOpType.bypass,
                                replica_groups=[list(range(W))],
                                ins=[a2a_ins[0][:, :, :].opt()],
                                outs=[a2a_outs[0][:, :, :].opt()],
                            )

            nc.sync.dma_start(out=wo_sb[:, :, :, :], in_=wo[:, :, :, :])
            nc.sync.dma_start(out=wsum_sb[:, :, :, :], in_=wsum[:, :, :, :])
            nc.sync.dma_start(out=h2_sb[:, :, :], in_=h2[:, :, :])
            nc.sync.dma_start(out=h2s_sb[:, :], in_=h2s[:, :])
            nc.sync.dma_start(out=obg[:, :, :], in_=obg_in[:, :, :])
            # ---- per-half output projection + residual + LayerNorm ----
            # half 0 = batch-0 tokens (y cols 0:HT), overlaps attn_b1;
            # half 1 = batch-1 tokens, after the second collective.
            TS = HT
            for hh in range(2):
                _mark(nc, f"tail{hh}")
                with ExitStack() as _op_stk:
                    op_ps = _op_stk.enter_context(
                        tc.tile_pool(name=f"op_ps{rep}_{hh}", bufs=2, space="PSUM"))
                    mom_ps = _op_stk.enter_context(
                        tc.tile_pool(name=f"mom_ps{rep}_{hh}", bufs=2, space="PSUM"))
                    hs = slice(hh * TS, (hh + 1) * TS)
                    ctx_own = cop.tile([128, NK, TS], FP8, tag=f"ctxown{hh}")
                    nc.sync.dma_start(
                        out=ctx_own[:, :, :],
                        in_=a2a_outs[hh][:, :, :].rearrange("i p m -> p i m"),
                    )
                    # mean via row-summed Wo (DoubleRow, row 0 is real)
                    mom1 = mom_ps.tile([16, TS], F32, tag="mom1")
                    for p in range(NKP):
                        nc.tensor.matmul(
                            mom1[:, :],
                            wsum_sb[:, p, :, :],
                            ctx_own[:, 2 * p:2 * p + 2, :],
                            start=(p == 0), stop=(p == NKP - 1),
                            perf_mode=DR,
                            skip_group_check=True,
                        )
                    mom2 = mom_ps.tile([1, TS], F32, tag="mom2")
                    xts = []
                    for o in range(NK):
                        ps = op_ps.tile([128, TS], F32, tag="op")
                        for p in range(NKP):
                            nc.tensor.matmul(
                                ps[:, :],
                                wo_sb[:, p, :, o * 128:(o + 1) * 128],
                                ctx_own[:, 2 * p:2 * p + 2, :],
                                start=(p == 0),
                                stop=(p == NKP - 1),
                                perf_mode=DR,
                            )
                        xt_o = xtl.tile([128, TS], F32, tag="xt_o")
                        nc.vector.scalar_tensor_tensor(
                            xt_o[:, :], ps[:, :], 1.0 / WS,
                            h2_sb[:, o, hs],
                            op0=mybir.AluOpType.mult, op1=mybir.AluOpType.add,
                        )
                        xts.append(xt_o)
                        sq = stp.tile([128, TS], BF16, tag="sq", bufs=2)
                        nc.gpsimd.tensor_mul(sq[:, :], xt_o[:, :], xt_o[:, :])
                        nc.tensor.matmul(
                            mom2[:, :], ones128[:, :], sq[:, :],
                            start=(o == 0), stop=(o == NK - 1),
                            skip_group_check=True,
                        )
                    # mean / var -> rstd
                    muZ = lnp.tile([1, TS], F32, tag="muz", bufs=2)
                    m2Z = lnp.tile([1, TS], F32, tag="m2z", bufs=2)
                    nc.vector.scalar_tensor_tensor(
                        muZ[:, :], mom1[0:1, :], 1.0 / (WS * H),
                        h2s_sb[:, hs],
                        op0=mybir.AluOpType.mult, op1=mybir.AluOpType.add,
                    )
                    nc.vector.tensor_copy(m2Z[:, :], mom2[:, :])
                    mu_b = lnp.tile([128, TS], F32, tag="mub", bufs=2)
                    m2_b = lnp.tile([128, TS], F32, tag="m2b", bufs=2)
                    nc.gpsimd.partition_broadcast(mu_b[:, :], muZ[:, :], channels=128)
                    nc.gpsimd.partition_broadcast(m2_b[:, :], m2Z[:, :], channels=128)
                    musq = lnp.tile([128, TS], F32, tag="musq", bufs=2)
                    nc.gpsimd.tensor_mul(musq[:, :], mu_b[:, :], mu_b[:, :])
                    vare = lnp.tile([128, TS], F32, tag="vare", bufs=2)
                    nc.vector.scalar_tensor_tensor(
                        vare[:, :], m2_b[:, :], 1e-12, musq[:, :],
                        op0=mybir.AluOpType.add,
                        op1=mybir.AluOpType.subtract,
                    )
                    rvar = lnp.tile([128, TS], F32, tag="rvar", bufs=2)
                    nc.vector.reciprocal_approx_fast(rvar[:, :], vare[:, :])
                    rstd = lnp.tile([128, TS], F32, tag="rstd", bufs=2)
                    nc.scalar.activation(
                        rstd[:, :], rvar[:, :],
                        mybir.ActivationFunctionType.Sqrt,
                    )
                    for o in range(NK):
                        eng = nc.gpsimd if (o % 2) else nc.vector
                        dcen = stp.tile([128, TS], F32, tag="dcen", bufs=4)
                        eng.tensor_sub(dcen[:, :], xts[o][:, :], mu_b[:, :])
                        en = stp.tile([128, TS], F32, tag="en", bufs=4)
                        eng.tensor_mul(en[:, :], dcen[:, :], rstd[:, :])
                        outt = stp.tile([128, TS], F32, tag="outt", bufs=4)
                        eng.tensor_scalar(
                            outt[:, :], en[:, :],
                            obg[:, o, 0:1], obg[:, o, 1:2],
                            op0=mybir.AluOpType.mult, op1=mybir.AluOpType.add,
                        )
                        nc.sync.dma_start(
                            out=y[o * 128:(o + 1) * 128, hs], in_=outt[:, :]
                        )
                if hh == 0:
                    _mark(nc, "a2aB")
                    if no_collective:
                        for i in range(W):
                            nc.sync.dma_start(
                                out=a2a_outs[1][i, :, :],
                                in_=a2a_ins[1][i, :, :],
                            )
                    else:
                        nc.gpsimd.collective_compute(
                            "AllToAll",
                            mybir.AluOpType.bypass,
                            replica_groups=[list(range(W))],
                            ins=[a2a_ins[1][:, :, :].opt()],
                            outs=[a2a_outs[1][:, :, :].opt()],
                        )

    _mark(nc, "end")
    nc.compile()
    return nc


def get_nc(has_mask=False):
    key = ("nc", has_mask)
    if key not in _NC_CACHE:
        _NC_CACHE[key] = build_nc(has_mask=has_mask)
    return _NC_CACHE[key]


def prepare_in_maps(inputs):
    hidden = np.asarray(inputs["hidden_states"], dtype=np.float32)
    mask = np.asarray(inputs["attention_mask"], dtype=np.float32)
    Wq = np.asarray(inputs["Wq"], dtype=np.float32)
    Wk = np.asarray(inputs["Wk"], dtype=np.float32)
    Wv = np.asarray(inputs["Wv"], dtype=np.float32)
    Wo = np.asarray(inputs["Wo"], dtype=np.float32)
    bq = np.asarray(inputs["bq"], dtype=np.float32)
    bk = np.asarray(inputs["bk"], dtype=np.float32)
    bv = np.asarray(inputs["bv"], dtype=np.float32)
    bo = np.asarray(inputs["bo"], dtype=np.float32)
    gamma = np.asarray(inputs["ln_gamma"], dtype=np.float32)
    beta = np.asarray(inputs["ln_beta"], dtype=np.float32)

    X = hidden.reshape(T, H)
    xT = np.ascontiguousarray(X.T)                      # [H, T] f32
    x8_np = xT.astype(E4)
    has_mask = bool(np.any(mask))
    maskT_np = np.ascontiguousarray(mask.reshape(B, S))

    def dr_weights(Wm):
        # [H, M] -> [128(part), NKP, 2(plane), M]; rows (2p+i)*128+q -> [q,p,i]
        Wq8 = (Wm * WS).astype(E4)
        M = Wm.shape[1]
        return np.ascontiguousarray(
            Wq8.reshape(NKP, 2, 128, M).transpose(2, 0, 1, 3))

    wo8 = dr_weights(Wo)
    # row sums of the quantized Wo (x64 scale), padded to 16 cols for DR
    wsum_f = wo8.astype(np.float32).sum(axis=3, keepdims=True)  # [128,NKP,2,1]
    wsum8 = np.zeros((128, NKP, 2, 16), dtype=E4)
    wsum8[:, :, :, 0:1] = wsum_f.astype(E4)

    in_maps = []
    HT = TOK // 2
    for c in range(W):
        csl = slice(CPC * c, CPC * (c + 1))
        # core c owns tokens [c*HT,(c+1)*HT) of EACH batch
        h2_np = np.concatenate(
            [xT[:, c * HT:(c + 1) * HT], xT[:, S + c * HT:S + (c + 1) * HT]],
            axis=1) + bo[:, None]
        # [H, TOK] -> [128, NK, TOK] so the SBUF load is one contiguous DMA
        h2_r = np.ascontiguousarray(
            h2_np.reshape(NK, 128, TOK).transpose(1, 0, 2))
        obg_np = np.ascontiguousarray(
            np.stack([gamma.reshape(NK, 128).T, beta.reshape(NK, 128).T],
                     axis=2)).astype(np.float32)
        im = {
            "x8": x8_np,
            "wq": dr_weights(Wq[:, csl]),
            "wk": dr_weights(Wk[:, csl]),
            "wv": dr_weights(Wv[:, csl]),
            "bq": np.ascontiguousarray(bq[csl]) * WS,
            "bk": np.ascontiguousarray(bk[csl]) * WS,
            "bv": np.ascontiguousarray(bv[csl]),
            "wo": wo8,
            "wsum": wsum8,
            "obg_in": obg_np,
            "h2": h2_r,
            "h2s": (h2_np.sum(axis=0, keepdims=True) / H).astype(np.float32),
        }
        if has_mask:
            im["maskT"] = maskT_np
        in_maps.append(im)
    return in_maps, has_mask


def kernel(**inputs):
    in_maps, has_mask = prepare_in_maps(inputs)
    nc = get_nc(has_mask=has_mask)
    res = run_bass_kernel_spmd(nc, in_maps, core_ids=list(range(W)))
    out_flat = np.empty((T, H), dtype=np.float32)
    HT = TOK // 2
    for c in range(W):
        yv = res.results[c]["y"]
        out_flat[c * HT:(c + 1) * HT, :] = yv[:, 0:HT].T
        out_flat[S + c * HT:S + (c + 1) * HT, :] = yv[:, HT:TOK].T
    return out_flat.reshape(B, S, H)
